# revision 11
# baseline (speedup 1.0000x reference)
"""Trainium2 Bass kernel for nn_MultiHeadAttention_31052613550603.

Sharding: tensor-parallel over heads. 16 heads / 8 cores = 2 heads per core.
Each core computes Q/K/V projections for its 2 heads, full (non-causal)
softmax attention, and its row-shard of the output projection Wo. The
all-reduce of the 8 partial outputs is done host-side (cheap numpy sum).

Per-core device layout (batch processed sequentially, b=0,1):
  xT        [D=1024, S=2048] fp16  (host-transposed embeddings)
  QT2,KT2   [128, 2048] fp16       rows 0:64 head0's Q^T/K^T, 64:128 head1's
  V2T       [128, 2048] fp16       V^T, transposed on-device (XBAR DMA) to
  v2n       [128, 16, 128] fp16    V in natural [k,d] layout, per k-tile
  scores    S^T[k,q] in PSUM fp32, exp on ACT -> PT fp16 [k,q]
  O^T       accumulated in PSUM over k-tiles:  O^T[d,q] += V^T P^T
  Z         column sums of PT via ones-vector matmul on accumulated PTsum
  out       O^T/Z (2 heads stacked = 128 rows) @ Wo[128 rows] -> partial out
"""

import os
import numpy as np

import concourse.bass as bass
import concourse.tile as tile
from concourse import bacc, mybir
from concourse.bass import ts
from concourse.bass_utils import run_bass_kernel_spmd

F16 = mybir.dt.float16
F32 = mybir.dt.float32
F32R = mybir.dt.float32r
EXP = mybir.ActivationFunctionType.Exp

B, S, D, H, DK, DV = 2, 2048, 1024, 16, 64, 64
NCORES = 8
HPC = H // NCORES          # heads per core = 2
D2 = HPC * DV              # 128, stacked head dim
QB = 512                   # q block (columns per attention pass)
NQB = S // QB              # 4
NKT = S // 128             # 16 k-tiles
MCH = D // 128             # 8 m-chunks for projections
NO_B = D // 512            # 2 n-blocks of output projection
SCALE = 1.0 / float(np.sqrt(DK))


def build(debug: bool = False, nrep: int = 1):
    nc = bacc.Bacc("TRN2", target_bir_lowering=False, debug=debug,
                   num_devices=NCORES)

    xT = nc.dram_tensor("xT", [B, D, S], F16, kind="ExternalInput").ap()
    wq2 = nc.dram_tensor("wq2", [D, D2], F16, kind="ExternalInput").ap()
    wk2 = nc.dram_tensor("wk2", [D, D2], F16, kind="ExternalInput").ap()
    wv2 = nc.dram_tensor("wv2", [D, D2], F16, kind="ExternalInput").ap()
    bq2 = nc.dram_tensor("bq2", [D2, 1], F32, kind="ExternalInput").ap()
    bk2 = nc.dram_tensor("bk2", [D2, 1], F32, kind="ExternalInput").ap()
    bv2 = nc.dram_tensor("bv2", [D2, 1], F32, kind="ExternalInput").ap()
    wo2 = nc.dram_tensor("wo2", [D2, D], F16, kind="ExternalInput").ap()
    ones1 = nc.dram_tensor("ones1", [128, 1], F16, kind="ExternalInput").ap()
    e2 = nc.dram_tensor("e2", [33, 128], F32, kind="ExternalInput").ap()
    out_p = nc.dram_tensor("out_p", [B * S, D], F16, kind="ExternalOutput").ap()

    from contextlib import ExitStack
    with tile.TileContext(nc) as tc, ExitStack() as ctx:
        consts = ctx.enter_context(tc.tile_pool(name="consts", bufs=1))
        xt_pool = ctx.enter_context(tc.tile_pool(name="xt", bufs=2))
        proj_pool = ctx.enter_context(tc.tile_pool(name="projT", bufs=2))
        v2n_pool = ctx.enter_context(tc.tile_pool(name="v2n", bufs=2))
        ot_pool = ctx.enter_context(tc.tile_pool(name="ot", bufs=2))
        pt_pool = ctx.enter_context(tc.tile_pool(name="pt", bufs=3))
        pts_pool = ctx.enter_context(tc.tile_pool(name="pts", bufs=2))
        z2_pool = ctx.enter_context(tc.tile_pool(name="z2", bufs=2))
        osb_pool = ctx.enter_context(tc.tile_pool(name="osb", bufs=3))
        # PSUM pools.  8 banks: st2 tiles are 2 banks each (bufs=2 -> 4),
        # psO 1 bank (bufs=2), shared [128,512] pool for proj/Zb/outproj.
        ps_st = ctx.enter_context(tc.tile_pool(name="ps_st", bufs=2, space="PSUM"))
        ps_o = ctx.enter_context(tc.tile_pool(name="ps_o", bufs=2, space="PSUM"))
        ps_mm = ctx.enter_context(tc.tile_pool(name="ps_mm", bufs=2, space="PSUM"))

        # ---- constants ----
        w_sb = {}
        for name, ap in (("q", wq2), ("k", wk2), ("v", wv2)):
            t = consts.tile([128, MCH, D2], F16, name=f"w_{name}")
            nc.sync.dma_start(t[:], ap.rearrange("(mo p) c -> p mo c", p=128))
            w_sb[name] = t
        b_sb = {}
        for name, ap in (("q", bq2), ("k", bk2), ("v", bv2)):
            t = consts.tile([D2, 1], F32, name=f"b_{name}")
            nc.sync.dma_start(t[:], ap)
            b_sb[name] = t
        wo_sb = consts.tile([D2, D], F16, name="wo")
        nc.sync.dma_start(wo_sb[:], wo2)
        ones_sb = consts.tile([128, 1], F16, name="ones")
        nc.sync.dma_start(ones_sb[:], ones1)
        e2_sb = consts.tile([33, 128], F32, name="e2")
        nc.sync.dma_start(e2_sb[:], e2)

        from contextlib import nullcontext
        rep_ctx = (tc.For_i(0, nrep, 1,
                            hint_engines=(mybir.EngineType.PE,
                                          mybir.EngineType.DVE,
                                          mybir.EngineType.Activation,
                                          mybir.EngineType.SP))
                   if nrep > 1 else nullcontext())
        with rep_ctx:
          for b in range(B):
            # ---- load x^T for this batch ----
            xt_sb = xt_pool.tile([128, MCH, S], F16, tag="xt")
            for m in range(MCH):
                nc.sync.dma_start(xt_sb[:, m, :], xT[b, m * 128:(m + 1) * 128, :])

            # ---- projections: K first, then V, then Q (attention q_blk 0
            # needs all of K,V but only the first q-block of Q) ----
            projT = {}
            for name in ("k", "v", "q"):
                dst = proj_pool.tile([D2, S], F16, tag=f"projT_{name}")
                projT[name] = dst
                for j in range(NQB):
                    ps = ps_mm.tile([128, 512], F32, tag="mm")
                    for m in range(MCH):
                        nc.tensor.matmul(ps[:], lhsT=w_sb[name][:, m, :],
                                         rhs=xt_sb[:, m, ts(j, 512)],
                                         start=(m == 0), stop=(m == MCH - 1))
                    nc.vector.tensor_scalar(dst[:, ts(j, 512)], ps[:],
                                            b_sb[name][:], None,
                                            op0=mybir.AluOpType.add)

            # ---- V -> natural layout via XBAR transpose ----
            v2n = v2n_pool.tile([128, NKT, D2], F16, tag="v2n")
            for t in range(NKT):
                nc.sync.dma_start_transpose(v2n[:, t, :], projT["v"][:, ts(t, 128)])

            # ---- attention ----
            ot_sb = ot_pool.tile([D2, S], F16, tag="ot")
            for j in range(NQB):
                pts2 = pts_pool.tile([128, 2 * QB], F16, tag="pts")
                nc.vector.memset(pts2[:], 0.0)
                psO = ps_o.tile([128, QB], F32, tag="psO")
                for t in range(NKT):
                    st2 = ps_st.tile([128, 2 * QB], F32, tag="st")
                    nc.tensor.matmul(st2[:, 0:QB],
                                     lhsT=projT["k"][0:64, ts(t, 128)],
                                     rhs=projT["q"][0:64, ts(j, QB)],
                                     start=True, stop=True)
                    nc.tensor.matmul(st2[:, QB:2 * QB],
                                     lhsT=projT["k"][64:128, ts(t, 128)],
                                     rhs=projT["q"][64:128, ts(j, QB)],
                                     start=True, stop=True)
                    pt2 = pt_pool.tile([128, 2 * QB], F16, tag="pt")
                    nc.scalar.activation(pt2[:], st2[:], EXP, scale=SCALE)
                    nc.tensor.matmul(psO[0:64, :], lhsT=v2n[:, t, 0:64],
                                     rhs=pt2[:, 0:QB],
                                     start=(t == 0), stop=(t == NKT - 1),
                                     skip_group_check=True)
                    nc.tensor.matmul(psO[64:128, :], lhsT=v2n[:, t, 64:128],
                                     rhs=pt2[:, QB:2 * QB],
                                     start=(t == 0), stop=(t == NKT - 1),
                                     skip_group_check=True)
                    nc.vector.tensor_add(pts2[:], pts2[:], pt2[:])

                # Z = column sums of PT per head; then 1/Z broadcast to the
                # 128 output rows via a tiny 2-row matmul.
                psZ = ps_mm.tile([128, 512], F32, tag="mm")
                nc.tensor.matmul(psZ[0:1, :], lhsT=ones_sb[:],
                                 rhs=pts2[:, 0:QB], start=True, stop=True,
                                 skip_group_check=True)
                nc.tensor.matmul(psZ[32:33, :], lhsT=ones_sb[:],
                                 rhs=pts2[:, QB:2 * QB], start=True,
                                 stop=True, skip_group_check=True)
                z2 = z2_pool.tile([33, QB], F32, tag="z2")
                nc.vector.memset(z2[:], 0.0)
                nc.vector.reciprocal(z2[0:1, :], psZ[0:1, :])
                nc.vector.reciprocal(z2[32:33, :], psZ[32:33, :])
                psZb = ps_mm.tile([128, 512], F32, tag="mm")
                nc.tensor.matmul(psZb[:], lhsT=e2_sb[:],
                                 rhs=z2[:], start=True, stop=True)
                zb_sb = z2_pool.tile([128, QB], F32, tag="zb")
                nc.vector.tensor_copy(zb_sb[:], psZb[:])
                nc.vector.tensor_mul(ot_sb[:, ts(j, QB)], psO[:], zb_sb[:])

                # ---- output projection for the 4 s-tiles of this q block ----
                for i in range(4 * j, 4 * j + 4):
                    for nb in range(NO_B):
                        pso = ps_mm.tile([128, 512], F32, tag="mm")
                        nc.tensor.matmul(pso[:], lhsT=ot_sb[:, ts(i, 128)],
                                         rhs=wo_sb[:, ts(nb, 512)],
                                         start=True, stop=True)
                        osb = osb_pool.tile([128, 512], F16, tag="osb")
                        nc.vector.tensor_copy(osb[:], pso[:])
                        nc.sync.dma_start(
                            out_p[b * S + i * 128: b * S + (i + 1) * 128,
                                  ts(nb, 512)], osb[:])

    nc.compile()
    return nc


_NC_CACHE = {}


def _get_nc():
    if "nc" not in _NC_CACHE:
        _NC_CACHE["nc"] = build()
    return _NC_CACHE["nc"]


def make_in_maps(embeddings, Wq, bq, Wk, bk, Wv, bv, Wo, bo):
    embeddings = np.asarray(embeddings, dtype=np.float32)
    Wq, Wk, Wv = (np.asarray(a, np.float32) for a in (Wq, Wk, Wv))
    bq, bk, bv = (np.asarray(a, np.float32) for a in (bq, bk, bv))
    Wo = np.asarray(Wo, np.float32)

    xT = np.ascontiguousarray(embeddings.transpose(0, 2, 1)).astype(np.float16)
    ones1 = np.ones((128, 1), np.float16)
    e2 = np.zeros((33, 128), np.float32)
    e2[0, 0:64] = 1.0
    e2[32, 64:128] = 1.0

    in_maps = []
    for c in range(NCORES):
        h0, h1 = HPC * c, HPC * c + 1
        in_maps.append({
            "xT": xT,
            "wq2": np.concatenate([Wq[h0], Wq[h1]], axis=1).astype(np.float16),
            "wk2": np.concatenate([Wk[h0], Wk[h1]], axis=1).astype(np.float16),
            "wv2": np.concatenate([Wv[h0], Wv[h1]], axis=1).astype(np.float16),
            "bq2": np.concatenate([bq[h0], bq[h1]])[:, None].astype(np.float32),
            "bk2": np.concatenate([bk[h0], bk[h1]])[:, None].astype(np.float32),
            "bv2": np.concatenate([bv[h0], bv[h1]])[:, None].astype(np.float32),
            "wo2": Wo[HPC * DV * c: HPC * DV * (c + 1), :].astype(np.float16),
            "ones1": ones1,
            "e2": e2,
        })
    return in_maps


def kernel(embeddings, Wq, bq, Wk, bk, Wv, bv, Wo, bo):
    nc = _get_nc()
    in_maps = make_in_maps(embeddings, Wq, bq, Wk, bk, Wv, bv, Wo, bo)
    res = run_bass_kernel_spmd(nc, in_maps, core_ids=list(range(NCORES)))
    acc = np.zeros((B * S, D), np.float32)
    for r in res.results:
        acc += r["out_p"].astype(np.float32)
    acc += np.asarray(bo, np.float32)[None, :]
    return acc.reshape(B, S, D)


# revision 12
# speedup vs baseline: 1.4297x; 1.4297x over previous
"""Trainium2 Bass kernel for nn_MultiHeadAttention_31052613550603.

Sharding: tensor-parallel over heads. 16 heads / 8 cores = 2 heads per core.
Each core computes Q/K/V projections for its 2 heads, full (non-causal)
softmax attention, and its row-shard of the output projection Wo. The
all-reduce of the 8 partial outputs is done host-side (cheap numpy sum).

Per-core device layout (batch processed sequentially, b=0,1):
  xT        [D=1024, S=2048] fp16  (host-transposed embeddings)
  QT2,KT2   [128, 2048] fp16       rows 0:64 head0's Q^T/K^T, 64:128 head1's
  V2T       [128, 2048] fp16       V^T, transposed on-device (XBAR DMA) to
  v2n       [128, 16, 128] fp16    V in natural [k,d] layout, per k-tile
  scores    S^T[k,q] in PSUM fp32, exp on ACT -> PT fp16 [k,q]
  O^T       accumulated in PSUM over k-tiles:  O^T[d,q] += V^T P^T
  Z         column sums of PT via ones-vector matmul on accumulated PTsum
  out       O^T/Z (2 heads stacked = 128 rows) @ Wo[128 rows] -> partial out
"""

import os
import numpy as np

import concourse.bass as bass
import concourse.tile as tile
from concourse import bacc, mybir
from concourse.bass import ts
from concourse.bass_utils import run_bass_kernel_spmd

F16 = mybir.dt.float16
F32 = mybir.dt.float32
F32R = mybir.dt.float32r
EXP = mybir.ActivationFunctionType.Exp

B, S, D, H, DK, DV = 2, 2048, 1024, 16, 64, 64
NCORES = 8
HPC = H // NCORES          # heads per core = 2
D2 = HPC * DV              # 128, stacked head dim
QB = 512                   # q block (columns per attention pass)
NQB = S // QB              # 4
NKT = S // 128             # 16 k-tiles
MCH = D // 128             # 8 m-chunks for projections
NO_B = D // 512            # 2 n-blocks of output projection
SCALE = 1.0 / float(np.sqrt(DK))


def build(debug: bool = False, nrep: int = 1):
    nc = bacc.Bacc("TRN2", target_bir_lowering=False, debug=debug,
                   num_devices=NCORES)

    xT = nc.dram_tensor("xT", [B, D, S], F16, kind="ExternalInput").ap()
    wq2 = nc.dram_tensor("wq2", [D, D2], F16, kind="ExternalInput").ap()
    wk2 = nc.dram_tensor("wk2", [D, D2], F16, kind="ExternalInput").ap()
    wv2 = nc.dram_tensor("wv2", [D, D2], F16, kind="ExternalInput").ap()
    bq2 = nc.dram_tensor("bq2", [D2, 1], F32, kind="ExternalInput").ap()
    bk2 = nc.dram_tensor("bk2", [D2, 1], F32, kind="ExternalInput").ap()
    bv2 = nc.dram_tensor("bv2", [D2, 1], F32, kind="ExternalInput").ap()
    wo2 = nc.dram_tensor("wo2", [D2, D], F16, kind="ExternalInput").ap()
    ones1 = nc.dram_tensor("ones1", [128, 1], F16, kind="ExternalInput").ap()
    e2 = nc.dram_tensor("e2", [33, 128], F32, kind="ExternalInput").ap()
    out_p = nc.dram_tensor("out_p", [B * S, D], F16, kind="ExternalOutput").ap()

    from contextlib import ExitStack
    with tile.TileContext(nc) as tc, ExitStack() as ctx:
        consts = ctx.enter_context(tc.tile_pool(name="consts", bufs=1))
        xt_pool = ctx.enter_context(tc.tile_pool(name="xt", bufs=2))
        proj_pool = ctx.enter_context(tc.tile_pool(name="projT", bufs=2))
        v2n_pool = ctx.enter_context(tc.tile_pool(name="v2n", bufs=2))
        ot_pool = ctx.enter_context(tc.tile_pool(name="ot", bufs=2))
        pt_pool = ctx.enter_context(tc.tile_pool(name="pt", bufs=3))
        pts_pool = ctx.enter_context(tc.tile_pool(name="pts", bufs=2))
        z2_pool = ctx.enter_context(tc.tile_pool(name="z2", bufs=2))
        osb_pool = ctx.enter_context(tc.tile_pool(name="osb", bufs=3))
        # PSUM pools.  8 banks: st2 tiles are 2 banks each (bufs=2 -> 4),
        # psO 1 bank (bufs=2), shared [128,512] pool for proj/Zb/outproj.
        ps_st = ctx.enter_context(tc.tile_pool(name="ps_st", bufs=2, space="PSUM"))
        ps_o = ctx.enter_context(tc.tile_pool(name="ps_o", bufs=2, space="PSUM"))
        ps_mm = ctx.enter_context(tc.tile_pool(name="ps_mm", bufs=2, space="PSUM"))

        # ---- constants ----
        w_sb = {}
        for name, ap in (("q", wq2), ("k", wk2), ("v", wv2)):
            t = consts.tile([128, MCH, D2], F16, name=f"w_{name}")
            nc.sync.dma_start(t[:], ap.rearrange("(mo p) c -> p mo c", p=128))
            w_sb[name] = t
        b_sb = {}
        for name, ap in (("q", bq2), ("k", bk2), ("v", bv2)):
            t = consts.tile([D2, 1], F32, name=f"b_{name}")
            nc.sync.dma_start(t[:], ap)
            b_sb[name] = t
        wo_sb = consts.tile([D2, D], F16, name="wo")
        nc.sync.dma_start(wo_sb[:], wo2)
        ones_sb = consts.tile([128, 1], F16, name="ones")
        nc.sync.dma_start(ones_sb[:], ones1)
        e2_sb = consts.tile([33, 128], F32, name="e2")
        nc.sync.dma_start(e2_sb[:], e2)

        from contextlib import nullcontext
        rep_ctx = (tc.For_i(0, nrep, 1,
                            hint_engines=(mybir.EngineType.PE,
                                          mybir.EngineType.DVE,
                                          mybir.EngineType.Activation,
                                          mybir.EngineType.SP))
                   if nrep > 1 else nullcontext())
        with rep_ctx:
          for b in range(B):
            # ---- load x^T for this batch ----
            xt_sb = xt_pool.tile([128, MCH, S], F16, tag="xt")
            for m in range(MCH):
                nc.sync.dma_start(xt_sb[:, m, :], xT[b, m * 128:(m + 1) * 128, :])

            # ---- projections: K first, then V, then Q (attention q_blk 0
            # needs all of K,V but only the first q-block of Q) ----
            projT = {}
            for name in ("k", "v", "q"):
                dst = proj_pool.tile([D2, S], F16, tag=f"projT_{name}")
                projT[name] = dst
                for j in range(NQB):
                    ps = ps_mm.tile([128, 512], F32, tag="mm")
                    for m in range(MCH):
                        nc.tensor.matmul(ps[:], lhsT=w_sb[name][:, m, :],
                                         rhs=xt_sb[:, m, ts(j, 512)],
                                         start=(m == 0), stop=(m == MCH - 1))
                    nc.vector.tensor_scalar(dst[:, ts(j, 512)], ps[:],
                                            b_sb[name][:], None,
                                            op0=mybir.AluOpType.add)

            # ---- V -> natural layout via XBAR transpose ----
            v2n = v2n_pool.tile([128, NKT, D2], F16, tag="v2n")
            for t in range(NKT):
                nc.sync.dma_start_transpose(v2n[:, t, :], projT["v"][:, ts(t, 128)])

            # ---- attention ----
            ot_sb = ot_pool.tile([D2, S], F16, tag="ot")
            for j in range(NQB):
                pts2 = pts_pool.tile([128, 2 * QB], F16, tag="pts")
                nc.vector.memset(pts2[:], 0.0)
                psO = ps_o.tile([128, QB], F32, tag="psO")
                for t in range(NKT):
                    st2 = ps_st.tile([128, 2 * QB], F32, tag="st")
                    nc.tensor.matmul(st2[:, 0:QB],
                                     lhsT=projT["k"][0:64, ts(t, 128)],
                                     rhs=projT["q"][0:64, ts(j, QB)],
                                     start=True, stop=True)
                    nc.tensor.matmul(st2[:, QB:2 * QB],
                                     lhsT=projT["k"][64:128, ts(t, 128)],
                                     rhs=projT["q"][64:128, ts(j, QB)],
                                     start=True, stop=True)
                    pt2 = pt_pool.tile([128, 2 * QB], F16, tag="pt")
                    nc.scalar.activation(pt2[:], st2[:], EXP, scale=SCALE)
                    nc.tensor.matmul(psO[0:64, :], lhsT=v2n[:, t, 0:64],
                                     rhs=pt2[:, 0:QB],
                                     start=(t == 0), stop=(t == NKT - 1),
                                     skip_group_check=True)
                    nc.tensor.matmul(psO[64:128, :], lhsT=v2n[:, t, 64:128],
                                     rhs=pt2[:, QB:2 * QB],
                                     start=(t == 0), stop=(t == NKT - 1),
                                     skip_group_check=True)
                    if t % 2 == 0:
                        nc.vector.tensor_add(pts2[:], pts2[:], pt2[:])
                    else:
                        nc.gpsimd.tensor_add(pts2[:], pts2[:], pt2[:])

                # Z = column sums of PT per head; then 1/Z broadcast to the
                # 128 output rows via a tiny 2-row matmul.
                psZ = ps_mm.tile([128, 512], F32, tag="mm")
                nc.tensor.matmul(psZ[0:1, :], lhsT=ones_sb[:],
                                 rhs=pts2[:, 0:QB], start=True, stop=True,
                                 skip_group_check=True)
                nc.tensor.matmul(psZ[32:33, :], lhsT=ones_sb[:],
                                 rhs=pts2[:, QB:2 * QB], start=True,
                                 stop=True, skip_group_check=True)
                z2 = z2_pool.tile([33, QB], F32, tag="z2")
                nc.vector.memset(z2[:], 0.0)
                nc.vector.reciprocal(z2[0:1, :], psZ[0:1, :])
                nc.vector.reciprocal(z2[32:33, :], psZ[32:33, :])
                psZb = ps_mm.tile([128, 512], F32, tag="mm")
                nc.tensor.matmul(psZb[:], lhsT=e2_sb[:],
                                 rhs=z2[:], start=True, stop=True)
                zb_sb = z2_pool.tile([128, QB], F32, tag="zb")
                nc.vector.tensor_copy(zb_sb[:], psZb[:])
                nc.vector.tensor_mul(ot_sb[:, ts(j, QB)], psO[:], zb_sb[:])

                # ---- output projection for the 4 s-tiles of this q block ----
                for i in range(4 * j, 4 * j + 4):
                    for nb in range(NO_B):
                        pso = ps_mm.tile([128, 512], F32, tag="mm")
                        nc.tensor.matmul(pso[:], lhsT=ot_sb[:, ts(i, 128)],
                                         rhs=wo_sb[:, ts(nb, 512)],
                                         start=True, stop=True)
                        osb = osb_pool.tile([128, 512], F16, tag="osb")
                        nc.vector.tensor_copy(osb[:], pso[:])
                        nc.sync.dma_start(
                            out_p[b * S + i * 128: b * S + (i + 1) * 128,
                                  ts(nb, 512)], osb[:])

    nc.compile()
    return nc


_NC_CACHE = {}


def _get_nc():
    if "nc" not in _NC_CACHE:
        _NC_CACHE["nc"] = build()
    return _NC_CACHE["nc"]


def make_in_maps(embeddings, Wq, bq, Wk, bk, Wv, bv, Wo, bo):
    embeddings = np.asarray(embeddings, dtype=np.float32)
    Wq, Wk, Wv = (np.asarray(a, np.float32) for a in (Wq, Wk, Wv))
    bq, bk, bv = (np.asarray(a, np.float32) for a in (bq, bk, bv))
    Wo = np.asarray(Wo, np.float32)

    xT = np.ascontiguousarray(embeddings.transpose(0, 2, 1)).astype(np.float16)
    ones1 = np.ones((128, 1), np.float16)
    e2 = np.zeros((33, 128), np.float32)
    e2[0, 0:64] = 1.0
    e2[32, 64:128] = 1.0

    in_maps = []
    for c in range(NCORES):
        h0, h1 = HPC * c, HPC * c + 1
        in_maps.append({
            "xT": xT,
            "wq2": np.concatenate([Wq[h0], Wq[h1]], axis=1).astype(np.float16),
            "wk2": np.concatenate([Wk[h0], Wk[h1]], axis=1).astype(np.float16),
            "wv2": np.concatenate([Wv[h0], Wv[h1]], axis=1).astype(np.float16),
            "bq2": np.concatenate([bq[h0], bq[h1]])[:, None].astype(np.float32),
            "bk2": np.concatenate([bk[h0], bk[h1]])[:, None].astype(np.float32),
            "bv2": np.concatenate([bv[h0], bv[h1]])[:, None].astype(np.float32),
            "wo2": Wo[HPC * DV * c: HPC * DV * (c + 1), :].astype(np.float16),
            "ones1": ones1,
            "e2": e2,
        })
    return in_maps


def kernel(embeddings, Wq, bq, Wk, bk, Wv, bv, Wo, bo):
    nc = _get_nc()
    in_maps = make_in_maps(embeddings, Wq, bq, Wk, bk, Wv, bv, Wo, bo)
    res = run_bass_kernel_spmd(nc, in_maps, core_ids=list(range(NCORES)))
    acc = np.zeros((B * S, D), np.float32)
    for r in res.results:
        acc += r["out_p"].astype(np.float32)
    acc += np.asarray(bo, np.float32)[None, :]
    return acc.reshape(B, S, D)
